# revision 8
# baseline (speedup 1.0000x reference)
"""Trainium2 Bass kernel for nn_BeamSearchDecoder.

Input: probs (64, 1024, 1024) f32.  Output: (decoded (64,1024) i32, lengths (64,) i32).

Strategy (pure data parallelism over batch, 8 batches/core):
  - Device (the heavy, 256MB-bound pass): per (b,t) row of 1024 classes,
    compute the top-8 values + indices with the vector engine's Max/MaxIndex
    instructions (exactly jax.lax.top_k tie semantics: value desc, index asc).
  - Host: the beam recurrence only ever consumes the top-8 of each row
    (any candidate outside it is dominated by >=8 earlier flat-index
    candidates), so an 8x8 candidate scan with the reference's flat-index
    (p*C + c) tie-break reproduces the scores bit-exactly.  The rare steps
    where f32 rounding lets a 9th-or-lower row value tie into the selection
    are detected conservatively (monotonicity of f32 subtraction) and
    recomputed from the full row.  Then backtrace beam 7, column-wise
    unique_consecutive, blank removal, stable compaction.
"""
import numpy as np

B, T, C = 64, 1024, 1024
K = 8
BLANK = 0
N_CORES = 8
BATCH_PER_CORE = B // N_CORES          # 8
ROWS_PER_CORE = BATCH_PER_CORE * T     # 8192
ROWS_PER_PART = 4                      # rows handled per partition per block
BLOCK_ROWS = 128 * ROWS_PER_PART       # 512 rows per block
N_BLOCKS = ROWS_PER_CORE // BLOCK_ROWS  # 16

_PROGRAM = None


N_SLOTS = 8  # in-flight input tiles (double-buffer depth)


def _build_program():
    from contextlib import ExitStack

    import concourse.bass as bass
    import concourse.mybir as mybir

    nc = bass.Bass(name="beam_top8")
    x = nc.declare_dram_parameter(
        "x", [N_BLOCKS, 128, ROWS_PER_PART * C], mybir.dt.float32, isOutput=False
    )
    RES_W = N_BLOCKS * ROWS_PER_PART * K  # 512
    vals_out = nc.declare_dram_parameter(
        "vals", [128, RES_W], mybir.dt.float32, isOutput=True
    )
    idx_out = nc.declare_dram_parameter(
        "idx", [128, RES_W], mybir.dt.uint32, isOutput=True
    )

    with ExitStack() as ctx:
        s_slot = [
            ctx.enter_context(nc.semaphore(f"s_slot{k}")) for k in range(N_SLOTS)
        ]
        s_cons = ctx.enter_context(nc.semaphore("s_cons"))
        s_dve = ctx.enter_context(nc.semaphore("s_dve"))
        s_out = ctx.enter_context(nc.semaphore("s_out"))
        tiles = [
            ctx.enter_context(
                nc.sbuf_tensor(f"tile{k}", [128, ROWS_PER_PART * C], mybir.dt.float32)
            )
            for k in range(N_SLOTS)
        ]
        vt = ctx.enter_context(nc.sbuf_tensor("vt", [128, RES_W], mybir.dt.float32))
        it = ctx.enter_context(nc.sbuf_tensor("it", [128, RES_W], mybir.dt.uint32))

        with nc.Block() as block:

            @block.gpsimd
            def _(g):
                for b in range(N_BLOCKS):
                    k = b % N_SLOTS
                    if b >= N_SLOTS:
                        # slot free once block b-N_SLOTS fully consumed by DVE
                        g.wait_ge(s_cons, b - N_SLOTS + 1)
                    g.dma_start(out=tiles[k][:, :], in_=x[b]).then_inc(s_slot[k], 16)

            @block.vector
            def _(v):
                for b in range(N_BLOCKS):
                    k = b % N_SLOTS
                    v.wait_ge(s_slot[k], 16 * (b // N_SLOTS + 1))
                    for j in range(ROWS_PER_PART):
                        row = tiles[k][:, C * j : C * (j + 1)]
                        o = (b * ROWS_PER_PART + j) * K
                        v.max(out=vt[:, o : o + K], in_=row).then_inc(s_dve, 1)
                    # DVE pipeline: max's SBUF write is only visible to a
                    # later op after its completion sem fires
                    v.wait_ge(s_dve, ROWS_PER_PART * (b + 1))
                    last = None
                    for j in range(ROWS_PER_PART):
                        row = tiles[k][:, C * j : C * (j + 1)]
                        o = (b * ROWS_PER_PART + j) * K
                        last = v.max_index(
                            out=it[:, o : o + K],
                            in_max=vt[:, o : o + K],
                            in_values=row,
                        )
                    last.then_inc(s_cons, 1)

            @block.sync
            def _(s):
                s.wait_ge(s_cons, N_BLOCKS)
                s.dma_start(out=vals_out[:], in_=vt[:, :]).then_inc(s_out, 16)
                s.dma_start(out=idx_out[:], in_=it[:, :]).then_inc(s_out, 16)
                s.wait_ge(s_out, 32)

    return nc


def _get_program():
    global _PROGRAM
    if _PROGRAM is None:
        _PROGRAM = _build_program()
    return _PROGRAM


def _device_top8(probs: np.ndarray, want_profile: bool = False):
    """Run the top-8 pass on 8 NeuronCores. Returns m (B,T,8) f32, c (B,T,8) i32."""
    from concourse.bass_utils import run_bass_kernel_spmd

    nc = _get_program()
    shards = [
        np.ascontiguousarray(
            probs[i * BATCH_PER_CORE : (i + 1) * BATCH_PER_CORE]
        ).reshape(N_BLOCKS, 128, ROWS_PER_PART * C)
        for i in range(N_CORES)
    ]
    in_maps = [{"x": s} for s in shards]
    res = run_bass_kernel_spmd(
        nc, in_maps, core_ids=list(range(N_CORES)), trace=want_profile
    )
    m = np.empty((B, T, K), dtype=np.float32)
    c = np.empty((B, T, K), dtype=np.int32)
    for i in range(N_CORES):
        sl = slice(i * BATCH_PER_CORE, (i + 1) * BATCH_PER_CORE)
        # device row (p, b*32 + j*8 + k) holds top-k of shard row b*512 + p*4 + j
        vals = res.results[i]["vals"].reshape(128, N_BLOCKS, ROWS_PER_PART, K)
        idx = res.results[i]["idx"].reshape(128, N_BLOCKS, ROWS_PER_PART, K)
        m[sl] = vals.transpose(1, 0, 2, 3).reshape(BATCH_PER_CORE, T, K)
        c[sl] = (
            idx.transpose(1, 0, 2, 3).astype(np.int32).reshape(BATCH_PER_CORE, T, K)
        )
    return m, c, res


def _host_decode(probs: np.ndarray, m: np.ndarray, c: np.ndarray):
    """Beam scan + backtrace + unique_consecutive + compaction, exact."""
    # --- beam scan over T with 8x8 candidates, reference tie-break ---
    scores = np.full((B, K), np.inf, dtype=np.float32)
    scores[:, 0] = 0.0
    parents = np.empty((T, B, K), dtype=np.int8)
    clss = np.empty((T, B, K), dtype=np.int32)
    bi = np.arange(B)[:, None]
    for t in range(T):
        cand = (scores[:, :, None] - m[:, t, None, :]).reshape(B, K * K)
        flatkey = (np.arange(K)[:, None] * C + c[:, t, None, :]).reshape(B, K * K)
        sel = np.lexsort((flatkey, cand), axis=1)[:, :K]
        cut = cand[bi, sel[:, 7:8]][:, 0]
        # conservative exactness check vs candidates outside the top-8
        q = scores - m[:, t, 7][:, None]
        bad = np.min(q, axis=1) <= cut
        parents[t] = (sel // K).astype(np.int8)
        clss[t] = c[bi, t, sel % K]
        new_scores = cand[bi, sel]
        if np.any(bad):
            for b in np.where(bad)[0]:
                cf = (scores[b][:, None] - probs[b, t][None, :]).reshape(-1)
                s8 = np.argsort(cf, kind="stable")[:K]
                parents[t, b] = (s8 // C).astype(np.int8)
                clss[t, b] = (s8 % C).astype(np.int32)
                new_scores[b] = cf[s8]
        scores = new_scores

    # --- backtrace beam K-1 ---
    seqs = np.empty((B, T), dtype=np.int32)
    e = np.full(B, K - 1, dtype=np.int64)
    bia = np.arange(B)
    for t in range(T - 1, -1, -1):
        seqs[:, t] = clss[t, bia, e]
        e = parents[t, bia, e].astype(np.int64)

    # --- unique_consecutive columns + blank removal + stable compaction ---
    diff = np.any(seqs[:, 1:] != seqs[:, :-1], axis=0)
    col_keep = np.concatenate([np.ones(1, dtype=bool), diff])
    keep = col_keep[None, :] & (seqs != BLANK)
    order = np.argsort(~keep, axis=-1, kind="stable")
    vals = np.take_along_axis(seqs, order, axis=-1)
    mm = np.take_along_axis(keep, order, axis=-1)
    decoded = np.where(mm, vals, -1).astype(np.int32)
    lengths = np.sum(keep, axis=-1).astype(np.int32)
    return decoded, lengths


def kernel(probs: np.ndarray):
    probs = np.ascontiguousarray(np.asarray(probs, dtype=np.float32))
    m, c, _ = _device_top8(probs)
    return _host_decode(probs, m, c)


# revision 12
# speedup vs baseline: 1.0420x; 1.0420x over previous
"""Trainium2 Bass kernel for nn_BeamSearchDecoder.

Input: probs (64, 1024, 1024) f32.  Output: (decoded (64,1024) i32, lengths (64,) i32).

Strategy (pure data parallelism over batch, 8 batches/core):
  - Device (the heavy, 256MB-bound pass): per (b,t) row of 1024 classes,
    compute the top-8 values + indices with the vector engine's Max/MaxIndex
    instructions (exactly jax.lax.top_k tie semantics: value desc, index asc).
  - Host: the beam recurrence only ever consumes the top-8 of each row
    (any candidate outside it is dominated by >=8 earlier flat-index
    candidates), so an 8x8 candidate scan with the reference's flat-index
    (p*C + c) tie-break reproduces the scores bit-exactly.  The rare steps
    where f32 rounding lets a 9th-or-lower row value tie into the selection
    are detected conservatively (monotonicity of f32 subtraction) and
    recomputed from the full row.  Then backtrace beam 7, column-wise
    unique_consecutive, blank removal, stable compaction.
"""
import numpy as np

B, T, C = 64, 1024, 1024
K = 8
BLANK = 0
N_CORES = 8
BATCH_PER_CORE = B // N_CORES          # 8
ROWS_PER_CORE = BATCH_PER_CORE * T     # 8192
ROWS_PER_PART = 4                      # rows handled per partition per block
BLOCK_ROWS = 128 * ROWS_PER_PART       # 512 rows per block
N_BLOCKS = ROWS_PER_CORE // BLOCK_ROWS  # 16

_PROGRAM = None


N_SLOTS = 8  # in-flight input tiles (double-buffer depth)


def _build_program():
    from contextlib import ExitStack

    import concourse.bass as bass
    import concourse.mybir as mybir

    nc = bass.Bass(name="beam_top8")
    x = nc.declare_dram_parameter(
        "x", [N_BLOCKS, 128, ROWS_PER_PART * C], mybir.dt.float32, isOutput=False
    )
    RES_W = N_BLOCKS * ROWS_PER_PART * K  # 512
    vals_out = nc.declare_dram_parameter(
        "vals", [128, RES_W], mybir.dt.float32, isOutput=True
    )
    idx_out = nc.declare_dram_parameter(
        "idx", [128, RES_W], mybir.dt.uint32, isOutput=True
    )

    with ExitStack() as ctx:
        s_slot = [
            [
                ctx.enter_context(nc.semaphore(f"s_slot{k}_{j}"))
                for j in range(ROWS_PER_PART)
            ]
            for k in range(N_SLOTS)
        ]
        s_cons = ctx.enter_context(nc.semaphore("s_cons"))
        s_dve = ctx.enter_context(nc.semaphore("s_dve"))
        s_out = ctx.enter_context(nc.semaphore("s_out"))
        tiles = [
            ctx.enter_context(
                nc.sbuf_tensor(f"tile{k}", [128, ROWS_PER_PART * C], mybir.dt.float32)
            )
            for k in range(N_SLOTS)
        ]
        vt = ctx.enter_context(nc.sbuf_tensor("vt", [128, RES_W], mybir.dt.float32))
        it = ctx.enter_context(nc.sbuf_tensor("it", [128, RES_W], mybir.dt.uint32))

        with nc.Block() as block:

            @block.gpsimd
            def _(g):
                for b in range(N_BLOCKS):
                    k = b % N_SLOTS
                    if b >= N_SLOTS:
                        # slot free once block b-N_SLOTS fully consumed by DVE
                        g.wait_ge(s_cons, b - N_SLOTS + 1)
                    # quarter-grained loads: DVE can start row-group j as
                    # soon as its quarter lands
                    for j in range(ROWS_PER_PART):
                        g.dma_start(
                            out=tiles[k][:, C * j : C * (j + 1)],
                            in_=x[b][:, C * j : C * (j + 1)],
                        ).then_inc(s_slot[k][j], 16)

            def _do_max_index(v, b):
                k = b % N_SLOTS
                last = None
                for j in range(ROWS_PER_PART):
                    row = tiles[k][:, C * j : C * (j + 1)]
                    o = (b * ROWS_PER_PART + j) * K
                    last = v.max_index(
                        out=it[:, o : o + K],
                        in_max=vt[:, o : o + K],
                        in_values=row,
                    )
                last.then_inc(s_cons, 1)

            @block.vector
            def _(v):
                for b in range(N_BLOCKS):
                    k = b % N_SLOTS
                    rnd = b // N_SLOTS
                    for j in range(ROWS_PER_PART):
                        row = tiles[k][:, C * j : C * (j + 1)]
                        o = (b * ROWS_PER_PART + j) * K
                        v.wait_ge(s_slot[k][j], 16 * (rnd + 1))
                        v.max(out=vt[:, o : o + K], in_=row).then_inc(s_dve, 1)
                    # software pipeline: run block b-1's index pass now; its
                    # max ops retired while block b's maxes streamed, so this
                    # wait (DVE write visibility) is already satisfied
                    if b > 0:
                        v.wait_ge(s_dve, ROWS_PER_PART * b)
                        _do_max_index(v, b - 1)
                v.wait_ge(s_dve, ROWS_PER_PART * N_BLOCKS)
                _do_max_index(v, N_BLOCKS - 1)

            @block.sync
            def _(s):
                s.wait_ge(s_cons, N_BLOCKS)
                s.dma_start(out=vals_out[:], in_=vt[:, :]).then_inc(s_out, 16)
                s.dma_start(out=idx_out[:], in_=it[:, :]).then_inc(s_out, 16)
                s.wait_ge(s_out, 32)

    return nc


def _get_program():
    global _PROGRAM
    if _PROGRAM is None:
        _PROGRAM = _build_program()
    return _PROGRAM


def _device_top8(probs: np.ndarray, want_profile: bool = False):
    """Run the top-8 pass on 8 NeuronCores. Returns m (B,T,8) f32, c (B,T,8) i32."""
    from concourse.bass_utils import run_bass_kernel_spmd

    nc = _get_program()
    shards = [
        np.ascontiguousarray(
            probs[i * BATCH_PER_CORE : (i + 1) * BATCH_PER_CORE]
        ).reshape(N_BLOCKS, 128, ROWS_PER_PART * C)
        for i in range(N_CORES)
    ]
    in_maps = [{"x": s} for s in shards]
    res = run_bass_kernel_spmd(
        nc, in_maps, core_ids=list(range(N_CORES)), trace=want_profile
    )
    m = np.empty((B, T, K), dtype=np.float32)
    c = np.empty((B, T, K), dtype=np.int32)
    for i in range(N_CORES):
        sl = slice(i * BATCH_PER_CORE, (i + 1) * BATCH_PER_CORE)
        # device row (p, b*32 + j*8 + k) holds top-k of shard row b*512 + p*4 + j
        vals = res.results[i]["vals"].reshape(128, N_BLOCKS, ROWS_PER_PART, K)
        idx = res.results[i]["idx"].reshape(128, N_BLOCKS, ROWS_PER_PART, K)
        m[sl] = vals.transpose(1, 0, 2, 3).reshape(BATCH_PER_CORE, T, K)
        c[sl] = (
            idx.transpose(1, 0, 2, 3).astype(np.int32).reshape(BATCH_PER_CORE, T, K)
        )
    return m, c, res


def _host_decode(probs: np.ndarray, m: np.ndarray, c: np.ndarray):
    """Beam scan + backtrace + unique_consecutive + compaction, exact."""
    # --- beam scan over T with 8x8 candidates, reference tie-break ---
    scores = np.full((B, K), np.inf, dtype=np.float32)
    scores[:, 0] = 0.0
    parents = np.empty((T, B, K), dtype=np.int8)
    clss = np.empty((T, B, K), dtype=np.int32)
    bi = np.arange(B)[:, None]
    for t in range(T):
        cand = (scores[:, :, None] - m[:, t, None, :]).reshape(B, K * K)
        flatkey = (np.arange(K)[:, None] * C + c[:, t, None, :]).reshape(B, K * K)
        sel = np.lexsort((flatkey, cand), axis=1)[:, :K]
        cut = cand[bi, sel[:, 7:8]][:, 0]
        # conservative exactness check vs candidates outside the top-8
        q = scores - m[:, t, 7][:, None]
        bad = np.min(q, axis=1) <= cut
        parents[t] = (sel // K).astype(np.int8)
        clss[t] = c[bi, t, sel % K]
        new_scores = cand[bi, sel]
        if np.any(bad):
            for b in np.where(bad)[0]:
                cf = (scores[b][:, None] - probs[b, t][None, :]).reshape(-1)
                s8 = np.argsort(cf, kind="stable")[:K]
                parents[t, b] = (s8 // C).astype(np.int8)
                clss[t, b] = (s8 % C).astype(np.int32)
                new_scores[b] = cf[s8]
        scores = new_scores

    # --- backtrace beam K-1 ---
    seqs = np.empty((B, T), dtype=np.int32)
    e = np.full(B, K - 1, dtype=np.int64)
    bia = np.arange(B)
    for t in range(T - 1, -1, -1):
        seqs[:, t] = clss[t, bia, e]
        e = parents[t, bia, e].astype(np.int64)

    # --- unique_consecutive columns + blank removal + stable compaction ---
    diff = np.any(seqs[:, 1:] != seqs[:, :-1], axis=0)
    col_keep = np.concatenate([np.ones(1, dtype=bool), diff])
    keep = col_keep[None, :] & (seqs != BLANK)
    order = np.argsort(~keep, axis=-1, kind="stable")
    vals = np.take_along_axis(seqs, order, axis=-1)
    mm = np.take_along_axis(keep, order, axis=-1)
    decoded = np.where(mm, vals, -1).astype(np.int32)
    lengths = np.sum(keep, axis=-1).astype(np.int32)
    return decoded, lengths


def kernel(probs: np.ndarray):
    probs = np.ascontiguousarray(np.asarray(probs, dtype=np.float32))
    m, c, _ = _device_top8(probs)
    return _host_decode(probs, m, c)
